# revision 28
# baseline (speedup 1.0000x reference)
"""Multi-head causal attention (B=2, S=4096, D=512, H=8, DK=64) on 8 TRN2
NeuronCores.

Sharding: batch x head-pair. Core c handles batch c//4, heads {2*(c%4),
2*(c%4)+1} end-to-end through attention; head mixing for the output
projection happens via an AllGather of transposed per-head outputs (oT)
within each batch's 4-core group, after which each core applies the full
Wo to its 1024-row sequence slice.

Per-core dataflow (everything "T" is d-major, i.e. feature dim on SBUF
partitions, which is what the PE matmul contraction needs). The whole
datapath is bf16 (casting DMAs on load) with fp32 PSUM accumulation —
rel err ~4e-3 vs the 2e-2 gate, validated against an fp64 reference;
bf16 PE transposes run 2x the fp32 rate and their PSUM evacuations get
the 2x DVE mode:
  QT/KT/VT via PE transpose (bf16) -> qT/kT = W^T @ XT, v = V @ Wv
  sT[t, sq] = k^T q (transposed scores; K=64 contraction, bf16)
  wT = exp(sT/8) via ScalarE straight out of PSUM, over causally-live
       columns only; the causal mask is a post-exp 0/1 multiply on the
       128x128 diagonal blocks (bf16 SBUF 2x DVE, off the scores->exp
       chain; zeroed weights drop out of the ones-column denominator);
       strictly-above-diagonal blocks are never computed
  oT_aug = [v | 1]^T @ wT accumulated over t-chunks in one PSUM bank;
       row 64 is the softmax denominator (no separate reduction pass)
  oT = oT_aug[:64] * (1/oT_aug[64]) broadcast via GPSIMD partition_broadcast
  oT (bf16) AllGathered within the 4-core batch group in three pieces
       (q-chunks 0-3 / 4-5 / 6-7), each fired as soon as its columns are
       done so only the last 0.25MB gather sits on the critical tail;
       out-proj rows are selected with partition_id-driven conditional
       DMAs + dynamic slices; y = oT_all^T @ Wo (bf16), stored fp32.

Engine budget (cost model, per core): PE ~181us (scores+oT accumulation
are the floor at 16.8M PSUM elements / 128 lanes each; fp32 input
transposes 41us), ScalarE ~153us (144 exp instructions over [128,1024]
PSUM groups), VectorE ~110us (PSUM evacuations), 3 collectives.
PSUM (8 banks): 2x alternating single-buffer score pools + 2 oT
accumulators + 2 double-buffered single-bank scratch tiles for the
transpose/projection/out-proj pipeline. oT columns are staged to the
DRAM bounce per chunk so each AllGather fires the moment its last chunk
normalizes; weight loads queue behind the first input slice.
The diagonal mask-add is one DVE op per group covering both heads via a
zero-stride (head-broadcast) mask AP. TimelineSim end-to-end: ~284us
(compute span ~218us; the rest is the cost model's pessimistic AllGather
floor plus conditional DMAs it cannot see are skipped).
"""

import sys

sys.path.insert(0, "/opt/trn_rl_repo")

import numpy as np

import concourse.bass as bass
import concourse.mybir as mybir
import concourse.tile as tile
from concourse import bacc
from concourse.bass import ds, ts
from concourse.bass_utils import run_bass_kernel_spmd
from concourse.masks import make_identity

B, S, D, H, DK = 2, 4096, 512, 8, 64
SQ, TC = 512, 128  # q-chunk rows, t-chunk rows
NSL = S // SQ  # 8 row slices
NCHUNK = S // TC  # 32 t-chunks
f32 = mybir.dt.float32
f32r = mybir.dt.float32r
bf16 = mybir.dt.bfloat16
AF = mybir.ActivationFunctionType
ALU = mybir.AluOpType

_CACHED_NC = None


def attention_chunk(nc, pool, sA, sB, otp, mask128, qt_sl, kt_sl, v_sl, ot_half,
                    bounce_in_part, crel, gi0):
    """Attention for q-chunk c, both heads, t-chunks 0..4(c+1)-1.

    Each scores/exp group holds one t-chunk for BOTH heads ([128, 2, 512]);
    oT accumulates per head in its own PSUM bank across the t loop."""
    c = len(qt_sl) - 1  # current q-chunk == latest slice
    n_tc = 4 * (c + 1)
    ot_ps = [None, None]
    for tcg in range(n_tc):
        r = tcg - 4 * c
        sl, lc = tcg // 4, tcg % 4
        n0 = 128 * r if r >= 0 else 0
        gi = gi0 + tcg
        sp = (sA if gi % 2 == 0 else sB).tile(
            [128, 2, 512], f32, tag="sA" if gi % 2 == 0 else "sB"
        )
        for h in range(2):
            nc.tensor.matmul(
                sp[:, h, n0:512],
                lhsT=kt_sl[sl][64 * h : 64 * h + 64, ts(lc, 128)],
                rhs=qt_sl[c][64 * h : 64 * h + 64, n0:512],
                start=True,
                stop=True,
            )
        wt = pool("wt", 6).tile([128, 2, 512], bf16, tag="wt")
        # diagonal groups: exp only the causally-live columns (cols < n0 are
        # stale PSUM never read by the oT matmuls below)
        nc.scalar.activation(wt[:, :, n0:512], sp[:, :, n0:512], AF.Exp, scale=0.125)
        if r >= 0:
            # causal mask as a post-exp 0/1 multiply on the diagonal block:
            # cheaper (bf16 SBUF 2x DVE mode vs fp32 PSUM add) and off the
            # scores->exp critical chain; the softmax denominator comes from
            # the ones-column oT sum below, so zeroed weights drop out of it
            mask2 = bass.AP(
                tensor=mask128.tensor,
                offset=mask128.offset,
                ap=[mask128.ap[0], [0, 2], [1, 128]],
            )
            nc.vector.tensor_mul(
                wt[:, :, n0 : n0 + 128], wt[:, :, n0 : n0 + 128], mask2
            )
        for h in range(2):
            if tcg == 0:
                ot_ps[h] = otp.tile([128, 512], f32, tag="otp", name=f"otp_c{c}h{h}")
            nc.tensor.matmul(
                ot_ps[h][0:65, n0:512],
                lhsT=v_sl[sl][:, lc, 65 * h : 65 * h + 65],
                rhs=wt[:, h, n0:512],
                start=(tcg == 0),
                stop=(tcg == n_tc - 1),
            )
    for h in range(2):
        # one cheap copy releases the PSUM accumulator immediately; the
        # normalize chain then runs off the oT-accumulation critical path
        ot_raw = pool("otraw", 4).tile([65, 512], f32, tag="otraw",
                                       name=f"otraw_c{c}h{h}")
        nc.vector.tensor_copy(ot_raw, ot_ps[h][0:65, :])
        recip = pool("recip", 2).tile([1, 512], f32, tag="recip")
        nc.vector.reciprocal(recip, ot_raw[64:65, :])
        rbc = pool("rbc", 2).tile([64, 512], f32, tag="rbc")
        nc.gpsimd.partition_broadcast(rbc, recip)
        nc.vector.tensor_mul(
            ot_half[64 * h : 64 * h + 64, crel, :], ot_raw[0:64, :], rbc
        )
    nc.sync.dma_start(
        bounce_in_part[:, ts(crel, 512)], ot_half[:, crel, :]
    )


def _build_body(nc, tc, Q, K, V, Wq, Wk, Wv, Wo, Y):
    ctx_pools = {}

    def pool(name, bufs, space="SBUF"):
        if name not in ctx_pools:
            ctx_pools[name] = tc.alloc_tile_pool(name=name, bufs=bufs, space=space)
        return ctx_pools[name]

    def psum_pool(name, bufs):
        return pool(name, bufs, space="PSUM")

    const = pool("const", 1)
    ident16 = const.tile([128, 128], bf16, tag="id16")
    make_identity(nc, ident16)
    # multiplicative causal mask for a 128x128 diagonal block: keep (1)
    # where col >= row, else 0 — applied to the exp'd weights
    mask128 = const.tile([128, 128], bf16, tag="mask")
    nc.vector.memset(mask128, 1.0)
    nc.gpsimd.affine_select(
        out=mask128,
        in_=mask128,
        compare_op=ALU.is_ge,
        fill=0.0,
        base=0,
        channel_multiplier=-1,
        pattern=[[1, 128]],
    )


    mm = psum_pool("mm", 2)  # [128, 512] single-bank tiles, double-buffered
    sA = psum_pool("sA", 1)  # [128, 2, 512] scores group (even)
    sB = psum_pool("sB", 1)  # [128, 2, 512] scores group (odd)
    otp = psum_pool("otp", 2)  # [128, 512] oT accumulator

    qt_sl, kt_sl, v_sl = [], [], []
    GI = [0]
    pid = nc.partition_id()
    hp = pid % 4
    ag_conds = [hp < 2, (hp > 1) & (hp < 3), hp > 2]
    off0 = (pid % 2) * 1024
    ot_all = [
        pool("otall", 4).tile([128, 1024], bf16, tag="otall", name=f"otall{dc}")
        for dc in range(4)
    ]

    # attention staging: heads paired per scores group (PE row-group
    # concurrency); oT written bf16, AllGathered in three overlapped pieces
    # (chunks 0-3 / 4-5 / 6-7) so only the last small AG sits on the tail
    # (a 4-way split with a chunk-7-only tail gather measured ~3us WORSE
    # on HW — the extra AllGather's fixed floor isn't hidden)
    AG_SPLIT = [(0, 4), (4, 6), (6, 8)]  # [c0, c1) chunk ranges
    ot_parts = [
        pool("ot", 1).tile([128, c1 - c0, 512], bf16, tag=f"otp{i}", name=f"otsb{i}")
        for i, (c0, c1) in enumerate(AG_SPLIT)
    ]
    dram = pool("dram", 1, space="DRAM")
    bounce_in = [
        dram.tile([128, (c1 - c0) * 512], bf16, tag=f"bin{i}", name=f"bin{i}")
        for i, (c0, c1) in enumerate(AG_SPLIT)
    ]
    bounce_out = [
        dram.tile([512, (c1 - c0) * 512], bf16, tag=f"bout{i}", name=f"bout{i}")
        for i, (c0, c1) in enumerate(AG_SPLIT)
    ]

    # ---------------- phase 1: load, transpose, project ----------------
    Qr = Q.rearrange("(s g p) d -> s p g d", p=128, g=4)
    Kr = K.rearrange("(s g p) d -> s p g d", p=128, g=4)
    Vr = V.rearrange("(s g p) d -> s p g d", p=128, g=4)

    wq_sb = wk_sb = wv_sb = wo_sb = None
    for s in range(NSL):
        # --- Q and K: bf16 path (casting DMAs; bf16 transposes run 2x the
        # fp32 PE rate and their PSUM evacuations get the 2x DVE mode; fp32
        # PSUM accumulation in every matmul keeps rel err ~4e-3) ---
        xq = pool("xin", 4).tile([128, 4, 512], bf16, tag="xin")
        nc.gpsimd.dma_start(xq, Qr[s])  # casting DMA f32 -> bf16
        xk = pool("xin", 4).tile([128, 4, 512], bf16, tag="xin")
        nc.gpsimd.dma_start(xk, Kr[s])
        if s == 0:
            # weight loads queued behind the first slice so they don't
            # delay the first transposes
            wq_sb = const.tile([128, 4, 128], bf16, tag="wq")
            nc.gpsimd.dma_start(wq_sb, Wq.rearrange("(c p) k -> p c k", p=128))
            wk_sb = const.tile([128, 4, 128], bf16, tag="wk")
            nc.gpsimd.dma_start(wk_sb, Wk.rearrange("(c p) k -> p c k", p=128))
            wv_sb = const.tile([128, 4, 128], bf16, tag="wv")
            nc.gpsimd.dma_start(wv_sb, Wv.rearrange("(c p) k -> p c k", p=128))
            wo_sb = const.tile([128, 4, 512], bf16, tag="wo")
            nc.gpsimd.dma_start(wo_sb, Wo.rearrange("(c p) n -> p c n", p=128))
        xtq = pool("xt", 3).tile([128, 4, 512], bf16, tag="xtqk")
        xtk = pool("xt", 3).tile([128, 4, 512], bf16, tag="xtqk")
        for x_sl, xt_sb in ((xq, xtq), (xk, xtk)):
            for dc in range(4):
                t_ps = mm.tile([128, 512], bf16, tag="mm", name=f"tps_{s}_{dc}")
                for g in range(4):
                    nc.tensor.transpose(
                        t_ps[:, ts(g, 128)], x_sl[:, g, ts(dc, 128)], ident16
                    )
                nc.vector.tensor_copy(xt_sb[:, dc, :], t_ps)
        # qT/kT projections (both heads of the pair): [128, 512]
        qt_ps = mm.tile([128, 512], f32, tag="mm")
        for dc in range(4):
            nc.tensor.matmul(
                qt_ps,
                lhsT=wq_sb[:, dc, :],
                rhs=xtq[:, dc, :],
                start=(dc == 0),
                stop=(dc == 3),
            )
        qt = pool("qt", NSL).tile([128, 512], bf16, tag="qt")
        nc.vector.tensor_copy(qt, qt_ps)
        qt_sl.append(qt)
        kt_ps = mm.tile([128, 512], f32, tag="mm")
        for dc in range(4):
            nc.tensor.matmul(
                kt_ps,
                lhsT=wk_sb[:, dc, :],
                rhs=xtk[:, dc, :],
                start=(dc == 0),
                stop=(dc == 3),
            )
        kt = pool("kt", NSL).tile([128, 512], bf16, tag="kt")
        nc.vector.tensor_copy(kt, kt_ps)
        kt_sl.append(kt)

        # --- V: bf16 path ---
        xv = pool("xinv", 2).tile([128, 4, 512], bf16, tag="xinv")
        nc.gpsimd.dma_start(xv, Vr[s])  # casting DMA f32 -> bf16
        xtv = pool("xtv", 3).tile([128, 4, 512], bf16, tag="xtv")
        for dc in range(4):
            t_ps = mm.tile([128, 512], bf16, tag="mm", name=f"tpsv_{s}_{dc}")
            for g in range(4):
                nc.tensor.transpose(
                    t_ps[:, ts(g, 128)], xv[:, g, ts(dc, 128)], ident16
                )
            nc.vector.tensor_copy(xtv[:, dc, :], t_ps)
        # v projection, t-major: per t-chunk [128, 2*64]; interleave into
        # v_aug [128, 4, 130] with a ones column per head at 65h+64
        vp = mm.tile([128, 512], f32, tag="mm")
        for tcl in range(4):
            for dc in range(4):
                nc.tensor.matmul(
                    vp[:, ts(tcl, 128)],
                    lhsT=xtv[:, dc, ts(tcl, 128)],
                    rhs=wv_sb[:, dc, :],
                    start=(dc == 0),
                    stop=(dc == 3),
                )
        va = pool("v", NSL).tile([128, 4, 130], bf16, tag="v")
        nc.vector.memset(va.rearrange("p c (h k) -> p c h k", k=65)[:, :, :, 64:65], 1.0)
        nc.vector.tensor_copy(
            va.rearrange("p c (h k) -> p c h k", k=65)[:, :, :, 0:64],
            vp.rearrange("p (c h k) -> p c h k", c=4, h=2),
        )
        v_sl.append(va)

        part = next(i for i, (c0, c1) in enumerate(AG_SPLIT) if c0 <= s < c1)
        attention_chunk(nc, pool, sA, sB, otp, mask128, qt_sl, kt_sl, v_sl,
                        ot_parts[part], bounce_in[part],
                        s - AG_SPLIT[part][0], GI[0])
        GI[0] += 4 * (s + 1)
        if s == AG_SPLIT[part][1] - 1:
            nc.gpsimd.collective_compute(
                "AllGather",
                ALU.bypass,
                replica_groups=[[0, 1, 2, 3], [4, 5, 6, 7]],
                ins=[bounce_in[part].opt()],
                outs=[bounce_out[part].opt()],
            )
            for dc in range(4):
                nc.sync.dma_start(
                    ot_all[dc],
                    bounce_out[part][ts(dc, 128), ds(off0, 1024)]
                    if part == 0
                    else bounce_out[part][ts(dc, 128), 0:1024],
                    cond=ag_conds[part],
                )

    # ------------- phase 3: gather my columns + output projection -------
    for st in range(8):
        ym = mm.tile([128, 512], f32, tag="mm", name=f"ym_{st}")
        for dc in range(4):
            nc.tensor.matmul(
                ym,
                lhsT=ot_all[dc][:, ts(st, 128)],
                rhs=wo_sb[:, dc, :],
                start=(dc == 0),
                stop=(dc == 3),
            )
        y_sb = pool("y", 4).tile([128, 512], f32, tag="y")
        if st % 2 == 0:
            nc.scalar.copy(y_sb, ym)
            nc.sync.dma_start(Y[ts(st, 128), :], y_sb)
        else:
            nc.vector.tensor_copy(y_sb, ym)
            nc.scalar.dma_start(Y[ts(st, 128), :], y_sb)

    for p in reversed(list(ctx_pools.values())):
        p.release()


def _build(loop=1):
    global _CACHED_NC
    if loop == 1 and _CACHED_NC is not None:
        return _CACHED_NC
    nc = bacc.Bacc("TRN2", num_devices=8)
    Q = nc.dram_tensor("Q", [S, D], f32, kind="ExternalInput")
    K = nc.dram_tensor("K", [S, D], f32, kind="ExternalInput")
    V = nc.dram_tensor("V", [S, D], f32, kind="ExternalInput")
    Wq = nc.dram_tensor("Wq", [D, 128], f32, kind="ExternalInput")
    Wk = nc.dram_tensor("Wk", [D, 128], f32, kind="ExternalInput")
    Wv = nc.dram_tensor("Wv", [D, 128], f32, kind="ExternalInput")
    Wo = nc.dram_tensor("Wo", [D, D], f32, kind="ExternalInput")
    Y = nc.dram_tensor("Y", [1024, D], f32, kind="ExternalOutput")
    with tile.TileContext(nc) as tcx:
        for _ in range(loop):
            _build_body(nc, tcx, Q, K, V, Wq, Wk, Wv, Wo, Y)
    nc.finalize()
    if loop == 1:
        _CACHED_NC = nc
    return nc


def _in_maps(inputs):
    Q, K, V = (np.asarray(inputs[k], np.float32) for k in ("Q", "K", "V"))
    Wq, Wk, Wv, Wo = (
        np.asarray(inputs[k], np.float32) for k in ("Wq", "Wk", "Wv", "Wo")
    )
    in_maps = []
    for c in range(8):
        b, hp = c // 4, c % 4
        in_maps.append(
            {
                "Q": np.ascontiguousarray(Q[b]),
                "K": np.ascontiguousarray(K[b]),
                "V": np.ascontiguousarray(V[b]),
                "Wq": np.ascontiguousarray(
                    np.concatenate([Wq[2 * hp], Wq[2 * hp + 1]], axis=1)
                ),
                "Wk": np.ascontiguousarray(
                    np.concatenate([Wk[2 * hp], Wk[2 * hp + 1]], axis=1)
                ),
                "Wv": np.ascontiguousarray(
                    np.concatenate([Wv[2 * hp], Wv[2 * hp + 1]], axis=1)
                ),
                "Wo": Wo,
            }
        )
    return in_maps


def _assemble(per_core_results):
    out = np.empty((B, S, D), np.float32)
    for c in range(8):
        b, hp = c // 4, c % 4
        out[b, 1024 * hp : 1024 * (hp + 1)] = per_core_results[c]["Y"]
    return out


def kernel(Q, K, V, Wq, Wk, Wv, Wo):
    nc = _build()
    in_maps = _in_maps(
        {"Q": Q, "K": K, "V": V, "Wq": Wq, "Wk": Wk, "Wv": Wv, "Wo": Wo}
    )
    res = run_bass_kernel_spmd(nc, in_maps, core_ids=list(range(8)))
    return _assemble(res.results)



# revision 30
# speedup vs baseline: 1.0050x; 1.0050x over previous
"""Multi-head causal attention (B=2, S=4096, D=512, H=8, DK=64) on 8 TRN2
NeuronCores.

Sharding: batch x head-pair. Core c handles batch c//4, heads {2*(c%4),
2*(c%4)+1} end-to-end through attention; head mixing for the output
projection happens via an AllGather of transposed per-head outputs (oT)
within each batch's 4-core group, after which each core applies the full
Wo to its 1024-row sequence slice.

Per-core dataflow (everything "T" is d-major, i.e. feature dim on SBUF
partitions, which is what the PE matmul contraction needs). The whole
datapath is bf16 (casting DMAs on load) with fp32 PSUM accumulation —
rel err ~4e-3 vs the 2e-2 gate, validated against an fp64 reference;
bf16 PE transposes run 2x the fp32 rate and their PSUM evacuations get
the 2x DVE mode:
  QT/KT/VT via PE transpose (bf16) -> qT/kT = W^T @ XT, v = V @ Wv
  sT[t, sq] = k^T q (transposed scores; K=64 contraction, bf16)
  wT = exp(sT/8) via ScalarE straight out of PSUM, over causally-live
       columns only; the causal mask is a post-exp 0/1 multiply on the
       128x128 diagonal blocks (bf16 SBUF 2x DVE, off the scores->exp
       chain; zeroed weights drop out of the ones-column denominator);
       strictly-above-diagonal blocks are never computed
  oT_aug = [v | 1]^T @ wT accumulated over t-chunks in one PSUM bank;
       row 64 is the softmax denominator (no separate reduction pass)
  oT = oT_aug[:64] * (1/oT_aug[64]) broadcast via GPSIMD partition_broadcast
  oT (bf16) AllGathered within the 4-core batch group in three pieces
       (q-chunks 0-3 / 4-5 / 6-7), each fired as soon as its columns are
       done so only the last 0.25MB gather sits on the critical tail;
       out-proj rows are selected with partition_id-driven conditional
       DMAs + dynamic slices; y = oT_all^T @ Wo (bf16), stored fp32.

Engine budget (cost model, per core): PE ~181us (scores+oT accumulation
are the floor at 16.8M PSUM elements / 128 lanes each; fp32 input
transposes 41us), ScalarE ~153us (144 exp instructions over [128,1024]
PSUM groups), VectorE ~110us (PSUM evacuations), 3 collectives.
PSUM (8 banks): 2x alternating single-buffer score pools + 2 oT
accumulators + 2 double-buffered single-bank scratch tiles for the
transpose/projection/out-proj pipeline. oT columns are staged to the
DRAM bounce per chunk so each AllGather fires the moment its last chunk
normalizes; weight loads queue behind the first input slice.
The diagonal mask-add is one DVE op per group covering both heads via a
zero-stride (head-broadcast) mask AP. TimelineSim end-to-end: ~284us
(compute span ~218us; the rest is the cost model's pessimistic AllGather
floor plus conditional DMAs it cannot see are skipped).
"""

import sys

sys.path.insert(0, "/opt/trn_rl_repo")

import numpy as np

import concourse.bass as bass
import concourse.mybir as mybir
import concourse.tile as tile
from concourse import bacc
from concourse.bass import ds, ts
from concourse.bass_utils import run_bass_kernel_spmd
from concourse.masks import make_identity

B, S, D, H, DK = 2, 4096, 512, 8, 64
SQ, TC = 512, 128  # q-chunk rows, t-chunk rows
NSL = S // SQ  # 8 row slices
NCHUNK = S // TC  # 32 t-chunks
f32 = mybir.dt.float32
f32r = mybir.dt.float32r
bf16 = mybir.dt.bfloat16
AF = mybir.ActivationFunctionType
ALU = mybir.AluOpType

_CACHED_NC = None


def attention_chunk(nc, pool, sA, sB, otp, mask128, qt_sl, kt_sl, v_sl, ot_half,
                    bounce_in_part, crel, gi0):
    """Attention for q-chunk c, both heads, t-chunks 0..4(c+1)-1.

    Each scores/exp group holds one t-chunk for BOTH heads ([128, 2, 512]);
    oT accumulates per head in its own PSUM bank across the t loop."""
    c = len(qt_sl) - 1  # current q-chunk == latest slice
    n_tc = 4 * (c + 1)
    ot_ps = [None, None]
    for tcg in range(n_tc):
        r = tcg - 4 * c
        sl, lc = tcg // 4, tcg % 4
        n0 = 128 * r if r >= 0 else 0
        gi = gi0 + tcg
        sp = (sA if gi % 2 == 0 else sB).tile(
            [128, 2, 512], f32, tag="sA" if gi % 2 == 0 else "sB"
        )
        for h in range(2):
            nc.tensor.matmul(
                sp[:, h, n0:512],
                lhsT=kt_sl[sl][64 * h : 64 * h + 64, ts(lc, 128)],
                rhs=qt_sl[c][64 * h : 64 * h + 64, n0:512],
                start=True,
                stop=True,
            )
        wt = pool("wt", 6).tile([128, 2, 512], bf16, tag="wt")
        # diagonal groups: exp only the causally-live columns (cols < n0 are
        # stale PSUM never read by the oT matmuls below)
        nc.scalar.activation(wt[:, :, n0:512], sp[:, :, n0:512], AF.Exp, scale=0.125)
        if r >= 0:
            # causal mask as a post-exp 0/1 multiply on the diagonal block:
            # cheaper (bf16 SBUF 2x DVE mode vs fp32 PSUM add) and off the
            # scores->exp critical chain; the softmax denominator comes from
            # the ones-column oT sum below, so zeroed weights drop out of it
            mask2 = bass.AP(
                tensor=mask128.tensor,
                offset=mask128.offset,
                ap=[mask128.ap[0], [0, 2], [1, 128]],
            )
            nc.vector.tensor_mul(
                wt[:, :, n0 : n0 + 128], wt[:, :, n0 : n0 + 128], mask2
            )
        for h in range(2):
            if tcg == 0:
                ot_ps[h] = otp.tile([128, 512], f32, tag="otp", name=f"otp_c{c}h{h}")
            nc.tensor.matmul(
                ot_ps[h][0:65, n0:512],
                lhsT=v_sl[sl][:, lc, 65 * h : 65 * h + 65],
                rhs=wt[:, h, n0:512],
                start=(tcg == 0),
                stop=(tcg == n_tc - 1),
            )
    for h in range(2):
        # one cheap copy releases the PSUM accumulator immediately; the
        # normalize chain then runs off the oT-accumulation critical path
        ot_raw = pool("otraw", 4).tile([65, 512], f32, tag="otraw",
                                       name=f"otraw_c{c}h{h}")
        nc.vector.tensor_copy(ot_raw, ot_ps[h][0:65, :])
        recip = pool("recip", 2).tile([1, 512], f32, tag="recip")
        nc.vector.reciprocal(recip, ot_raw[64:65, :])
        rbc = pool("rbc", 2).tile([64, 512], f32, tag="rbc")
        nc.gpsimd.partition_broadcast(rbc, recip)
        nc.vector.tensor_mul(
            ot_half[64 * h : 64 * h + 64, crel, :], ot_raw[0:64, :], rbc
        )
    nc.sync.dma_start(
        bounce_in_part[:, ts(crel, 512)], ot_half[:, crel, :]
    )


def _build_body(nc, tc, Q, K, V, Wq, Wk, Wv, Wo, Y):
    ctx_pools = {}

    def pool(name, bufs, space="SBUF"):
        if name not in ctx_pools:
            ctx_pools[name] = tc.alloc_tile_pool(name=name, bufs=bufs, space=space)
        return ctx_pools[name]

    def psum_pool(name, bufs):
        return pool(name, bufs, space="PSUM")

    const = pool("const", 1)
    ident16 = const.tile([128, 128], bf16, tag="id16")
    make_identity(nc, ident16)
    # multiplicative causal mask for a 128x128 diagonal block: keep (1)
    # where col >= row, else 0 — applied to the exp'd weights
    mask128 = const.tile([128, 128], bf16, tag="mask")
    nc.vector.memset(mask128, 1.0)
    nc.gpsimd.affine_select(
        out=mask128,
        in_=mask128,
        compare_op=ALU.is_ge,
        fill=0.0,
        base=0,
        channel_multiplier=-1,
        pattern=[[1, 128]],
    )


    mm = psum_pool("mm", 2)  # [128, 512] single-bank tiles, double-buffered
    sA = psum_pool("sA", 1)  # [128, 2, 512] scores group (even)
    sB = psum_pool("sB", 1)  # [128, 2, 512] scores group (odd)
    otp = psum_pool("otp", 2)  # [128, 512] oT accumulator

    qt_sl, kt_sl, v_sl = [], [], []
    GI = [0]
    pid = nc.partition_id()
    hp = pid % 4
    ag_conds = [hp < 2, (hp > 1) & (hp < 3), hp > 2]
    off0 = (pid % 2) * 1024
    ot_all = [
        pool("otall", 4).tile([128, 1024], bf16, tag="otall", name=f"otall{dc}")
        for dc in range(4)
    ]

    # attention staging: heads paired per scores group (PE row-group
    # concurrency); oT written bf16, AllGathered in three overlapped pieces
    # (chunks 0-3 / 4-5 / 6-7) so only the last small AG sits on the tail
    # (a 4-way split with a chunk-7-only tail gather measured ~3us WORSE
    # on HW — the extra AllGather's fixed floor isn't hidden)
    AG_SPLIT = [(0, 4), (4, 6), (6, 8)]  # [c0, c1) chunk ranges
    ot_parts = [
        pool("ot", 1).tile([128, c1 - c0, 512], bf16, tag=f"otp{i}", name=f"otsb{i}")
        for i, (c0, c1) in enumerate(AG_SPLIT)
    ]
    dram = pool("dram", 1, space="DRAM")
    bounce_in = [
        dram.tile([128, (c1 - c0) * 512], bf16, tag=f"bin{i}", name=f"bin{i}")
        for i, (c0, c1) in enumerate(AG_SPLIT)
    ]
    bounce_out = [
        dram.tile([512, (c1 - c0) * 512], bf16, tag=f"bout{i}", name=f"bout{i}")
        for i, (c0, c1) in enumerate(AG_SPLIT)
    ]

    # ---------------- phase 1: load, transpose, project ----------------
    Qr = Q.rearrange("(s g p) d -> s p g d", p=128, g=4)
    Kr = K.rearrange("(s g p) d -> s p g d", p=128, g=4)
    Vr = V.rearrange("(s g p) d -> s p g d", p=128, g=4)

    wq_sb = wk_sb = wv_sb = wo_sb = None
    for s in range(NSL):
        # --- Q and K: bf16 path (casting DMAs; bf16 transposes run 2x the
        # fp32 PE rate and their PSUM evacuations get the 2x DVE mode; fp32
        # PSUM accumulation in every matmul keeps rel err ~4e-3) ---
        xq = pool("xin", 4).tile([128, 4, 512], bf16, tag="xin")
        nc.gpsimd.dma_start(xq, Qr[s])  # casting DMA f32 -> bf16
        xk = pool("xin", 4).tile([128, 4, 512], bf16, tag="xin")
        nc.gpsimd.dma_start(xk, Kr[s])
        if s == 0:
            # weight loads queued behind the first slice so they don't
            # delay the first transposes
            wq_sb = const.tile([128, 4, 128], bf16, tag="wq")
            nc.gpsimd.dma_start(wq_sb, Wq.rearrange("(c p) k -> p c k", p=128))
            wk_sb = const.tile([128, 4, 128], bf16, tag="wk")
            nc.gpsimd.dma_start(wk_sb, Wk.rearrange("(c p) k -> p c k", p=128))
            wv_sb = const.tile([128, 4, 128], bf16, tag="wv")
            nc.gpsimd.dma_start(wv_sb, Wv.rearrange("(c p) k -> p c k", p=128))
            wo_sb = const.tile([128, 4, 512], bf16, tag="wo")
            nc.gpsimd.dma_start(wo_sb, Wo.rearrange("(c p) n -> p c n", p=128))
        xtq = pool("xt", 3).tile([128, 4, 512], bf16, tag="xtqk")
        xtk = pool("xt", 3).tile([128, 4, 512], bf16, tag="xtqk")
        for x_sl, xt_sb in ((xq, xtq), (xk, xtk)):
            for dc in range(4):
                t_ps = mm.tile([128, 512], bf16, tag="mm", name=f"tps_{s}_{dc}")
                for g in range(4):
                    nc.tensor.transpose(
                        t_ps[:, ts(g, 128)], x_sl[:, g, ts(dc, 128)], ident16
                    )
                nc.vector.tensor_copy(xt_sb[:, dc, :], t_ps)
        # qT/kT projections (both heads of the pair): [128, 512]
        qt_ps = mm.tile([128, 512], f32, tag="mm")
        for dc in range(4):
            nc.tensor.matmul(
                qt_ps,
                lhsT=wq_sb[:, dc, :],
                rhs=xtq[:, dc, :],
                start=(dc == 0),
                stop=(dc == 3),
            )
        qt = pool("qt", NSL).tile([128, 512], bf16, tag="qt")
        nc.vector.tensor_copy(qt, qt_ps)
        qt_sl.append(qt)
        kt_ps = mm.tile([128, 512], f32, tag="mm")
        for dc in range(4):
            nc.tensor.matmul(
                kt_ps,
                lhsT=wk_sb[:, dc, :],
                rhs=xtk[:, dc, :],
                start=(dc == 0),
                stop=(dc == 3),
            )
        kt = pool("kt", NSL).tile([128, 512], bf16, tag="kt")
        nc.vector.tensor_copy(kt, kt_ps)
        kt_sl.append(kt)

        # --- V: bf16 path ---
        xv = pool("xinv", 2).tile([128, 4, 512], bf16, tag="xinv")
        nc.gpsimd.dma_start(xv, Vr[s])  # casting DMA f32 -> bf16
        xtv = pool("xtv", 3).tile([128, 4, 512], bf16, tag="xtv")
        for dc in range(4):
            t_ps = mm.tile([128, 512], bf16, tag="mm", name=f"tpsv_{s}_{dc}")
            for g in range(4):
                nc.tensor.transpose(
                    t_ps[:, ts(g, 128)], xv[:, g, ts(dc, 128)], ident16
                )
            nc.vector.tensor_copy(xtv[:, dc, :], t_ps)
        # v projection, t-major: per t-chunk [128, 2*64]; interleave into
        # v_aug [128, 4, 130] with a ones column per head at 65h+64
        vp = mm.tile([128, 512], f32, tag="mm")
        for tcl in range(4):
            for dc in range(4):
                nc.tensor.matmul(
                    vp[:, ts(tcl, 128)],
                    lhsT=xtv[:, dc, ts(tcl, 128)],
                    rhs=wv_sb[:, dc, :],
                    start=(dc == 0),
                    stop=(dc == 3),
                )
        va = pool("v", NSL).tile([128, 4, 130], bf16, tag="v")
        nc.vector.memset(va.rearrange("p c (h k) -> p c h k", k=65)[:, :, :, 64:65], 1.0)
        nc.vector.tensor_copy(
            va.rearrange("p c (h k) -> p c h k", k=65)[:, :, :, 0:64],
            vp.rearrange("p (c h k) -> p c h k", c=4, h=2),
        )
        v_sl.append(va)

        part = next(i for i, (c0, c1) in enumerate(AG_SPLIT) if c0 <= s < c1)
        attention_chunk(nc, pool, sA, sB, otp, mask128, qt_sl, kt_sl, v_sl,
                        ot_parts[part], bounce_in[part],
                        s - AG_SPLIT[part][0], GI[0])
        GI[0] += 4 * (s + 1)
        if s == AG_SPLIT[part][1] - 1:
            nc.gpsimd.collective_compute(
                "AllGather",
                ALU.bypass,
                replica_groups=[[0, 1, 2, 3], [4, 5, 6, 7]],
                ins=[bounce_in[part].opt()],
                outs=[bounce_out[part].opt()],
            )
            for dc in range(4):
                nc.sync.dma_start(
                    ot_all[dc],
                    bounce_out[part][ts(dc, 128), ds(off0, 1024)]
                    if part == 0
                    else bounce_out[part][ts(dc, 128), 0:1024],
                    cond=ag_conds[part],
                )

    # ------------- phase 3: gather my columns + output projection -------
    for st in range(8):
        ym = mm.tile([128, 512], f32, tag="mm", name=f"ym_{st}")
        for dc in range(4):
            nc.tensor.matmul(
                ym,
                lhsT=ot_all[dc][:, ts(st, 128)],
                rhs=wo_sb[:, dc, :],
                start=(dc == 0),
                stop=(dc == 3),
            )
        y_sb = pool("y", 4).tile([128, 512], f32, tag="y")
        if st % 2 == 0:
            nc.scalar.copy(y_sb, ym)
            nc.sync.dma_start(Y[ts(st, 128), :], y_sb)
        else:
            nc.vector.tensor_copy(y_sb, ym)
            nc.scalar.dma_start(Y[ts(st, 128), :], y_sb)

    for p in reversed(list(ctx_pools.values())):
        p.release()


def _build(loop=1):
    global _CACHED_NC
    if loop == 1 and _CACHED_NC is not None:
        return _CACHED_NC
    nc = bacc.Bacc("TRN2", num_devices=8)
    Q = nc.dram_tensor("Q", [S, D], f32, kind="ExternalInput")
    K = nc.dram_tensor("K", [S, D], f32, kind="ExternalInput")
    V = nc.dram_tensor("V", [S, D], f32, kind="ExternalInput")
    Wq = nc.dram_tensor("Wq", [D, 128], f32, kind="ExternalInput")
    Wk = nc.dram_tensor("Wk", [D, 128], f32, kind="ExternalInput")
    Wv = nc.dram_tensor("Wv", [D, 128], f32, kind="ExternalInput")
    Wo = nc.dram_tensor("Wo", [D, D], f32, kind="ExternalInput")
    Y = nc.dram_tensor("Y", [1024, D], f32, kind="ExternalOutput")
    with tile.TileContext(nc) as tcx:
        for _ in range(loop):
            _build_body(nc, tcx, Q, K, V, Wq, Wk, Wv, Wo, Y)
    nc.finalize()
    if loop == 1:
        _CACHED_NC = nc
    return nc


def _in_maps(inputs):
    Q, K, V = (np.asarray(inputs[k], np.float32) for k in ("Q", "K", "V"))
    Wq, Wk, Wv, Wo = (
        np.asarray(inputs[k], np.float32) for k in ("Wq", "Wk", "Wv", "Wo")
    )
    in_maps = []
    for c in range(8):
        b, hp = c // 4, c % 4
        in_maps.append(
            {
                "Q": np.ascontiguousarray(Q[b]),
                "K": np.ascontiguousarray(K[b]),
                "V": np.ascontiguousarray(V[b]),
                "Wq": np.ascontiguousarray(
                    np.concatenate([Wq[2 * hp], Wq[2 * hp + 1]], axis=1)
                ),
                "Wk": np.ascontiguousarray(
                    np.concatenate([Wk[2 * hp], Wk[2 * hp + 1]], axis=1)
                ),
                "Wv": np.ascontiguousarray(
                    np.concatenate([Wv[2 * hp], Wv[2 * hp + 1]], axis=1)
                ),
                "Wo": Wo,
            }
        )
    return in_maps


def _assemble(per_core_results):
    out = np.empty((B, S, D), np.float32)
    for c in range(8):
        b, hp = c // 4, c % 4
        out[b, 1024 * hp : 1024 * (hp + 1)] = per_core_results[c]["Y"]
    return out


def kernel(Q, K, V, Wq, Wk, Wv, Wo):
    nc = _build()
    in_maps = _in_maps(
        {"Q": Q, "K": K, "V": V, "Wq": Wq, "Wk": Wk, "Wv": Wv, "Wo": Wo}
    )
    res = run_bass_kernel_spmd(nc, in_maps, core_ids=list(range(8)))
    return _assemble(res.results)

